# revision 1
# baseline (speedup 1.0000x reference)
import numpy as np

B = 4
N_SPL = 1024
NF = 16
T = N_SPL + NF
T_LOC = T // 2
DIM = 1024
NH = 16
DH = 64
DEPTH = 6
FF = 2730
FF_PAD = 2816
FFC = FF_PAD // 128
VCOLS = NH * (DH + 1)
VOCAB = 20000
EMB_EXT = VOCAB + NF

KT_SZ = DIM * T_LOC
V_SZ = T_LOC * VCOLS
PACK = KT_SZ + V_SZ

TCH = [(0, 128), (128, 128), (256, 128), (384, 128), (512, 8)]
TC = [(0, 512), (512, 8)]
JCH = [(0, 0, 128), (0, 128, 128), (0, 256, 128), (0, 384, 128), (0, 512, 8),
       (1, 0, 128), (1, 128, 128), (1, 256, 128), (1, 384, 120)]
VC = [(0, 512), (512, 512), (1024, 16)]

_CACHE = {}


def _split_multi_waits(nc):
    import bass_rust
    import concourse.mybir as mybir

    for bb in nc.main_func.blocks:
        insts = list(bb.instructions)
        out = []
        changed = False
        for inst in insts:
            si = getattr(inst, "sync_info", None)
            waits = list(si.on_wait) if si is not None and si.on_wait else []
            if len(waits) > 1:
                for k, w in enumerate(waits[:-1]):
                    nop = bass_rust.InstNoOp(
                        name=f"{inst.name}-wsplit{k}",
                        engine=inst.engine,
                        sync_info=mybir.SyncInfo(on_wait=[w], on_update=[]),
                    )
                    nc.register_instruction(nop)
                    out.append(nop)
                inst.sync_info = mybir.SyncInfo(
                    on_wait=[waits[-1]], on_update=list(si.on_update or [])
                )
                changed = True
            out.append(inst)
        if changed:
            bb.instructions = out


def build_program(depth=DEPTH):
    import os
    skip = set(os.environ.get("KSKIP", "").split(","))
    import concourse.bass as bass
    import concourse.mybir as mybir
    import concourse.tile as tile

    F32 = mybir.dt.float32
    F32R = mybir.dt.float32r
    BF16 = mybir.dt.bfloat16
    I32 = mybir.dt.int32
    AF = mybir.ActivationFunctionType
    OP = mybir.AluOpType
    AX = mybir.AxisListType

    nc = bass.Bass()

    tokr = nc.declare_dram_parameter("tokr", [T_LOC, DIM], F32R, isOutput=False)
    tokw = nc.declare_dram_parameter("tokw", [T_LOC, 1], F32, isOutput=False)
    consts = nc.declare_dram_parameter("consts", [128, 128], F32R, isOutput=False)
    eyein = nc.declare_dram_parameter("eyein", [128, 128], F32R, isOutput=False)
    fmaskin = nc.declare_dram_parameter("fmaskin", [128, 1], F32R, isOutput=False)
    wq = nc.declare_dram_parameter("wq", [depth, DIM, DIM], BF16, isOutput=False)
    wk = nc.declare_dram_parameter("wk", [depth, DIM, DIM], BF16, isOutput=False)
    wv = nc.declare_dram_parameter("wv", [depth, DIM, VCOLS], BF16, isOutput=False)
    wo = nc.declare_dram_parameter("wo", [depth, DIM, DIM], BF16, isOutput=False)
    w1x = nc.declare_dram_parameter("w1x", [depth, DIM, FF_PAD], BF16, isOutput=False)
    w1g = nc.declare_dram_parameter("w1g", [depth, DIM, FF_PAD], BF16, isOutput=False)
    w2 = nc.declare_dram_parameter("w2", [depth, FF_PAD, DIM], BF16, isOutput=False)
    g1in = nc.declare_dram_parameter("g1in", [depth, 1, DIM], F32R, isOutput=False)
    g2in = nc.declare_dram_parameter("g2in", [depth, 1, DIM], F32R, isOutput=False)
    gfin = nc.declare_dram_parameter("gfin", [1, DIM], F32R, isOutput=False)
    osum = nc.declare_dram_parameter("osum", [DIM], F32, isOutput=True)

    rg = [[0, 1], [2, 3], [4, 5], [6, 7]]

    from contextlib import ExitStack

    with tile.TileContext(nc) as tc, nc.allow_low_precision(reason="tf32 compute"), \
         ExitStack() as es:
        outer = es.enter_context(tc.tile_pool(name="outer", bufs=1))
        dram = es.enter_context(tc.tile_pool(name="dram", bufs=1, space="DRAM"))

        cst = outer.tile([128, 128], F32R, tag="cst", name="cst")
        eye = outer.tile([128, 128], F32R, tag="eye", name="eye")
        fmask = outer.tile([128, 1], F32R, tag="fmask", name="fmask")
        nc.sync.dma_start(cst[:], consts[:])
        nc.sync.dma_start(eye[:], eyein[:])
        nc.sync.dma_start(fmask[:], fmaskin[:])
        scratch = outer.tile([128, T_LOC], F32R, tag="scratch", name="scratch")

        xt = [outer.tile([128, T_LOC], F32R, tag=f"xt{d}", name=f"xt{d}") for d in range(8)]

        with tc.tile_pool(name="emb_sb", bufs=2) as embp, \
             tc.tile_pool(name="emb_ps", bufs=2, space="PSUM") as embps:
            for ci, (toff, tsz) in enumerate(TCH):
                wt = embp.tile([128, 1], F32, tag=f"tw", name=f"tw")
                nc.sync.dma_start(wt[0:tsz, :], tokw[toff:toff + tsz, :])
                g = embp.tile([128, DIM], F32R, tag="gath", name="gath")
                nc.sync.dma_start(g[0:tsz, :], tokr[toff:toff + tsz, :])
                nc.vector.tensor_scalar_mul(g[0:tsz, :], g[0:tsz, :], wt[0:tsz, :])
                for dc in range(8):
                    pt = embps.tile([128, 128], F32R, tag="tp", name="tp")
                    nc.tensor.transpose(
                        pt[:, 0:tsz], g[0:tsz, dc * 128:(dc + 1) * 128],
                        eye[0:tsz, 0:tsz],
                    )
                    nc.scalar.copy(xt[dc][:, toff:toff + tsz], pt[:, 0:tsz])

        gf_row = outer.tile([1, DIM], F32R, tag="gfrow", name="gfrow")
        nc.sync.dma_start(gf_row[:], gfin[:])

        def layer_norm(src_tiles, dst_tiles, grow, sqp):
            if "ln" in skip:
                for dc in range(8):
                    nc.scalar.copy(dst_tiles[dc][:], src_tiles[dc][:])
                return
            with tc.tile_pool(name="ps_ln", bufs=1, space="PSUM") as psln:
                layer_norm_body(src_tiles, dst_tiles, grow, sqp, psln)

        def layer_norm_body(src_tiles, dst_tiles, grow, sqp, psln):
            mean_ps = psln.tile([1, T_LOC], F32, tag="mean", name="mean")
            sq_ps = psln.tile([1, T_LOC], F32, tag="sq", name="sq")
            for dc in range(8):
                sq_sb = sqp.tile([128, T_LOC], F32R, tag="sqt", name="sqt")
                nc.scalar.activation(sq_sb[:], src_tiles[dc][:], AF.Square)
                for (toff, tsz) in TC:
                    nc.tensor.matmul(
                        mean_ps[:, toff:toff + tsz], cst[:, 64:65],
                        src_tiles[dc][:, toff:toff + tsz],
                        start=(dc == 0), stop=(dc == 7),
                    )
                    nc.tensor.matmul(
                        sq_ps[:, toff:toff + tsz], cst[:, 64:65],
                        sq_sb[:, toff:toff + tsz],
                        start=(dc == 0), stop=(dc == 7),
                    )
            mean_sb = sqp.tile([1, T_LOC], F32R, tag="meansb", name="meansb")
            nc.scalar.copy(mean_sb[:], mean_ps[:])
            m2 = sqp.tile([1, T_LOC], F32, tag="m2", name="m2")
            nc.scalar.activation(m2[:], mean_ps[:], AF.Square)
            var = sqp.tile([1, T_LOC], F32, tag="var", name="var")
            nc.vector.tensor_tensor(out=var[:], in0=sq_ps[:], in1=m2[:], op=OP.subtract)
            srt = sqp.tile([1, T_LOC], F32, tag="srt", name="srt")
            nc.scalar.activation(srt[:], var[:], AF.Sqrt, bias=cst[0:1, 65:66])
            rstd = sqp.tile([1, T_LOC], F32R, tag="rstd", name="rstd")
            nc.vector.reciprocal(rstd[:], srt[:])
            mr = sqp.tile([1, T_LOC], F32R, tag="mr", name="mr")
            nc.vector.tensor_tensor(out=mr[:], in0=mean_sb[:], in1=rstd[:], op=OP.mult)
            for dc in range(8):
                g1_ps = psln.tile([128, T_LOC], F32, tag="G1", name="G1")
                g2_ps = psln.tile([128, T_LOC], F32, tag="G2", name="G2")
                for (toff, tsz) in TC:
                    nc.tensor.matmul(
                        g1_ps[:, toff:toff + tsz],
                        grow[0:1, dc * 128:(dc + 1) * 128],
                        rstd[0:1, toff:toff + tsz], start=True, stop=True,
                    )
                    nc.tensor.matmul(
                        g2_ps[:, toff:toff + tsz],
                        grow[0:1, dc * 128:(dc + 1) * 128],
                        mr[0:1, toff:toff + tsz], start=True, stop=True,
                    )
                tmp = sqp.tile([128, T_LOC], F32R, tag="lntmp", name="lntmp")
                nc.vector.tensor_tensor(out=tmp[:], in0=src_tiles[dc][:], in1=g1_ps[:], op=OP.mult)
                nc.vector.tensor_tensor(out=dst_tiles[dc][:], in0=tmp[:], in1=g2_ps[:], op=OP.subtract)

        for l in range(depth):
            with tc.tile_pool(name="qa", bufs=1) as qa:
                qt = [qa.tile([128, T_LOC], F32R, tag=f"qt{m}", name=f"qt{m}") for m in range(8)]
                att = [qa.tile([128, T_LOC], F32R, tag=f"att{g}", name=f"att{g}") for g in range(8)]
                bounce = dram.tile([PACK], F32R, tag="bounce", name="bounce")
                gath = dram.tile([2 * PACK], F32R, tag="gath", name="gath")

                with tc.tile_pool(name="xnw", bufs=1) as xnp, \
                     tc.tile_pool(name="wstr", bufs=2) as wstr, \
                     tc.tile_pool(name="sqp", bufs=2) as sqp:
                    xn = [xnp.tile([128, T_LOC], F32R, tag=f"xn{d}", name=f"xn{d}") for d in range(8)]
                    g1row = xnp.tile([1, DIM], F32R, tag="g1row", name="g1row")
                    nc.sync.dma_start(g1row[:], g1in[l])
                    layer_norm(xt, xn, g1row, sqp)
                    ps_qkv = es_qkv = None
                    from contextlib import ExitStack as _ES
                    es_qkv = _ES(); ps_qkv = es_qkv.enter_context(
                        tc.tile_pool(name="ps_qkv", bufs=2, space="PSUM"))

                    for mc in (range(8) if "qkv" not in skip else []):
                        wqt = wstr.tile([128, 8, 128], F32R, tag="wqt", name="wqt")
                        nc.gpsimd.dma_start(
                            wqt[:],
                            wq[l][:, mc * 128:(mc + 1) * 128]
                            .rearrange("(d p) c -> p d c", p=128))
                        qp = ps_qkv.tile([128, T_LOC], F32, tag="qk_ps", name="qk_ps")
                        for dc in range(8):
                            for (toff, tsz) in TC:
                                nc.tensor.matmul(
                                    qp[:, toff:toff + tsz], wqt[:, dc, :],
                                    xn[dc][:, toff:toff + tsz],
                                    start=(dc == 0), stop=(dc == 7),
                                )
                        nc.scalar.copy(qt[mc][:], qp[:])
                        wkt = wstr.tile([128, 8, 128], F32R, tag="wkt", name="wkt")
                        nc.gpsimd.dma_start(
                            wkt[:],
                            wk[l][:, mc * 128:(mc + 1) * 128]
                            .rearrange("(d p) c -> p d c", p=128))
                        kp = ps_qkv.tile([128, T_LOC], F32, tag="qk_ps", name="qk_ps")
                        for dc in range(8):
                            for (toff, tsz) in TC:
                                nc.tensor.matmul(
                                    kp[:, toff:toff + tsz], wkt[:, dc, :],
                                    xn[dc][:, toff:toff + tsz],
                                    start=(dc == 0), stop=(dc == 7),
                                )
                        ks = sqp.tile([128, T_LOC], F32R, tag="kstage", name="kstage")
                        nc.scalar.copy(ks[:], kp[:])
                        dst = bounce[mc * 128 * T_LOC:(mc + 1) * 128 * T_LOC]
                        nc.sync.dma_start(
                            dst.rearrange("(p t) -> p t", t=T_LOC), ks[:])
                    vs_t = [wstr.tile([128, VCOLS], F32R, tag=f"vs{c}", name=f"vs{c}", bufs=1)
                            for c in range(5)]
                    for (voff, vsz) in (VC if "qkv" not in skip else []):
                        wvt = wstr.tile([128, 8, 512], F32R, tag="wvt", name="wvt")
                        nc.gpsimd.dma_start(
                            wvt[:, :, 0:vsz],
                            wv[l][:, voff:voff + vsz]
                            .rearrange("(d p) c -> p d c", p=128))
                        for ci, (toff, tsz) in enumerate(TCH):
                            vp = ps_qkv.tile([128, 512], F32, tag="v_ps", name="v_ps")
                            for dc in range(8):
                                nc.tensor.matmul(
                                    vp[0:tsz, 0:vsz],
                                    xn[dc][:, toff:toff + tsz],
                                    wvt[:, dc, 0:vsz],
                                    start=(dc == 0), stop=(dc == 7),
                                )
                            nc.scalar.copy(vs_t[ci][0:tsz, voff:voff + vsz],
                                           vp[0:tsz, 0:vsz])
                    for ci, (toff, tsz) in enumerate(TCH):
                        dst = bounce[KT_SZ + toff * VCOLS: KT_SZ + (toff + tsz) * VCOLS]
                        nc.sync.dma_start(
                            dst.rearrange("(p t) -> p t", t=VCOLS), vs_t[ci][0:tsz, :])
                    es_qkv.close()

                nc.gpsimd.collective_compute(
                    "AllGather", mybir.AluOpType.bypass,
                    replica_groups=rg,
                    ins=[bounce.opt()], outs=[gath.opt()],
                )

                with tc.tile_pool(name="kv", bufs=1) as kvp, \
                     tc.tile_pool(name="attw", bufs=2) as attw:
                    ktg = [[kvp.tile([128, T_LOC], F32R, tag=f"ktg{r}_{m}", name=f"ktg{r}_{m}")
                            for m in range(8)] for r in range(2)]
                    vg = [[kvp.tile([128, VCOLS], F32R, tag=f"vg{r}_{c}", name=f"vg{r}_{c}")
                           for c in range(5)] for r in range(2)]
                    vfus = kvp.tile([16, VCOLS], F32R, tag="vfus", name="vfus")
                    for r in range(2):
                        base = r * PACK
                        for m in range(8):
                            src = gath[base + m * 128 * T_LOC: base + (m + 1) * 128 * T_LOC]
                            nc.sync.dma_start(
                                ktg[r][m][:], src.rearrange("(p t) -> p t", t=T_LOC))
                        for ci, (toff, tsz) in enumerate(TCH):
                            src = gath[base + KT_SZ + toff * VCOLS:
                                       base + KT_SZ + (toff + tsz) * VCOLS]
                            nc.sync.dma_start(
                                vg[r][ci][0:tsz, :],
                                src.rearrange("(p t) -> p t", t=VCOLS))
                            v3 = vg[r][ci][0:tsz, :].rearrange("p (g c) -> p g c", c=DH + 1)
                            nc.scalar.copy(v3[:, :, DH:DH + 1], cst[0:tsz, 0:NH].unsqueeze(-1))
                    src = gath[PACK + KT_SZ + 504 * VCOLS: PACK + KT_SZ + 520 * VCOLS]
                    nc.sync.dma_start(vfus[:], src.rearrange("(p t) -> p t", t=VCOLS))
                    vf3 = vfus[:].rearrange("p (g c) -> p g c", c=DH + 1)
                    nc.scalar.copy(vf3[:, :, DH:DH + 1], cst[0:16, 0:NH].unsqueeze(-1))

                    fus_sb = []
                    with tc.tile_pool(name="ps_fus", bufs=2, space="PSUM") as psF:
                        for h in (range(NH) if "attn" not in skip else []):
                            g, pb = h // 2, (h % 2) * 64
                            sf = psF.tile([16, 16], F32, tag="fus_s", name="fus_s")
                            nc.tensor.matmul(
                                sf[:], ktg[1][g][pb:pb + 64, 504:520],
                                qt[g][pb:pb + 64, 504:520], start=True, stop=True)
                            ef = attw.tile([16, 16], F32R, tag="fus_e", name="fus_e")
                            nc.scalar.activation(ef[:], sf[:], AF.Exp, scale=0.125)
                            of = psF.tile([65, 16], F32, tag="fus_o", name="fus_o")
                            nc.tensor.matmul(
                                of[:], vfus[:, h * 65:(h + 1) * 65], ef[:],
                                start=True, stop=True)
                            ft = kvp.tile([65, 16], F32, tag=f"fus{h}", name=f"fus{h}")
                            nc.scalar.copy(ft[:], of[:])
                            fus_sb.append(ft)

                    from contextlib import ExitStack as _ES
                    es_att = _ES()
                    psS5 = es_att.enter_context(tc.tile_pool(name="ps_s5", bufs=2, space="PSUM"))
                    psS8 = es_att.enter_context(tc.tile_pool(name="ps_s8", bufs=1, space="PSUM"))
                    psO5 = es_att.enter_context(tc.tile_pool(name="ps_o5", bufs=2, space="PSUM"))
                    psO8 = es_att.enter_context(tc.tile_pool(name="ps_o8", bufs=1, space="PSUM"))
                    psD = es_att.enter_context(tc.tile_pool(name="ps_d", bufs=1, space="PSUM"))
                    for h in (range(NH) if "attn" not in skip else []):
                        g, pb = h // 2, (h % 2) * 64
                        ou512 = psO5.tile([65, 512], F32, tag="ou512", name="ou512")
                        ou8 = psO8.tile([65, 8], F32, tag="ou8", name="ou8")
                        for ji, (r, joff, jsz) in enumerate(JCH):
                            first, last = (ji == 0), (ji == len(JCH) - 1)
                            ex = attw.tile([128, T_LOC], F32R, tag="exp", name="exp")
                            s_a = psS5.tile([128, 512], F32, tag="simA", name="simA")
                            s_b = psS8.tile([128, 8], F32, tag="simB", name="simB")
                            nc.tensor.matmul(
                                s_a[0:jsz, :], ktg[r][g][pb:pb + 64, joff:joff + jsz],
                                qt[g][pb:pb + 64, 0:512], start=True, stop=True)
                            nc.tensor.matmul(
                                s_b[0:jsz, :], ktg[r][g][pb:pb + 64, joff:joff + jsz],
                                qt[g][pb:pb + 64, 512:520], start=True, stop=True)
                            nc.scalar.activation(ex[0:jsz, 0:512], s_a[0:jsz, :], AF.Exp, scale=0.125)
                            nc.scalar.activation(ex[0:jsz, 512:520], s_b[0:jsz, :], AF.Exp, scale=0.125)
                            vtile = vg[r][joff // 128]
                            nc.tensor.matmul(
                                ou512[:], vtile[0:jsz, h * 65:(h + 1) * 65],
                                ex[0:jsz, 0:512], start=first, stop=last)
                            nc.tensor.matmul(
                                ou8[:], vtile[0:jsz, h * 65:(h + 1) * 65],
                                ex[0:jsz, 512:520], start=first, stop=last)
                        ou_sb = attw.tile([65, T_LOC], F32, tag="ousb", name="ousb")
                        nc.scalar.copy(ou_sb[:, 0:512], ou512[:])
                        nc.scalar.copy(ou_sb[:, 512:520], ou8[:])
                        nc.vector.scalar_tensor_tensor(
                            out=ou_sb[:, 504:520], in0=fus_sb[h][:],
                            scalar=fmask[0:65, 0:1], in1=ou_sb[:, 504:520],
                            op0=OP.mult, op1=OP.add)
                        nc.vector.reciprocal(scratch[64:65, :], ou_sb[64:65, :])
                        db512 = psD.tile([64, 512], F32, tag="db512", name="db512")
                        db8 = psD.tile([64, 8], F32, tag="db8", name="db8")
                        nc.tensor.matmul(db512[:], cst[64:65, 0:64],
                                         scratch[64:65, 0:512], start=True, stop=True)
                        nc.tensor.matmul(db8[:], cst[64:65, 0:64],
                                         scratch[64:65, 512:520], start=True, stop=True)
                        db_sb = attw.tile([64, T_LOC], F32, tag="dbsb", name="dbsb")
                        nc.scalar.copy(db_sb[:, 0:512], db512[:])
                        nc.scalar.copy(db_sb[:, 512:520], db8[:])
                        if h % 2 == 0:
                            nc.vector.tensor_tensor(
                                out=att[g][0:64, :], in0=ou_sb[0:64, :],
                                in1=db_sb[:], op=OP.mult)
                        else:
                            tmp = attw.tile([64, T_LOC], F32R, tag="atmp", name="atmp")
                            nc.vector.tensor_tensor(
                                out=tmp[:], in0=ou_sb[0:64, :], in1=db_sb[:], op=OP.mult)
                            nc.scalar.copy(att[g][64:128, :], tmp[:])
                    es_att.close()

                with tc.tile_pool(name="wop", bufs=2) as wop, \
                     tc.tile_pool(name="ps_wo", bufs=2, space="PSUM") as psW:
                    for fc in (range(8) if "wo" not in skip else []):
                        wot = wop.tile([128, 8, 128], F32R, tag="wot", name="wot")
                        nc.gpsimd.dma_start(
                            wot[:],
                            wo[l][:, fc * 128:(fc + 1) * 128]
                            .rearrange("(d p) c -> p d c", p=128))
                        pp = psW.tile([128, 512], F32, tag="proj5", name="proj5")
                        p8 = psW.tile([128, 8], F32, tag="proj8", name="proj8")
                        for ic in range(8):
                            nc.tensor.matmul(
                                pp[:], wot[:, ic, :],
                                att[ic][:, 0:512], start=(ic == 0), stop=(ic == 7))
                            nc.tensor.matmul(
                                p8[:], wot[:, ic, :],
                                att[ic][:, 512:520], start=(ic == 0), stop=(ic == 7))
                        nc.vector.tensor_tensor(
                            out=xt[fc][:, 0:512], in0=xt[fc][:, 0:512], in1=pp[:], op=OP.add)
                        nc.vector.tensor_tensor(
                            out=xt[fc][:, 512:520], in0=xt[fc][:, 512:520], in1=p8[:], op=OP.add)

            with tc.tile_pool(name="ff", bufs=1) as ffp, \
                 tc.tile_pool(name="ffw", bufs=3) as ffw, \
                 tc.tile_pool(name="sq2", bufs=2) as sqp2:
                xn2 = [ffp.tile([128, T_LOC], F32R, tag=f"xn2_{d}", name=f"xn2_{d}") for d in range(8)]
                g2row = ffp.tile([1, DIM], F32R, tag="g2row", name="g2row")
                nc.sync.dma_start(g2row[:], g2in[l])
                layer_norm(xt, xn2, g2row, sqp2)

                ffa = [ffp.tile([128, T_LOC], F32R, tag=f"ffa{m}", name=f"ffa{m}") for m in range(FFC)]
                from contextlib import ExitStack as _ES
                es_ff1 = _ES()
                psg2 = es_ff1.enter_context(
                    tc.tile_pool(name="ps_ff1", bufs=2, space="PSUM"))
                for mc in (range(FFC) if "ff1" not in skip else []):
                    w1xt = ffw.tile([128, 8, 128], F32R, tag="w1xt", name="w1xt")
                    w1gt = ffw.tile([128, 8, 128], F32R, tag="w1gt", name="w1gt")
                    nc.gpsimd.dma_start(
                        w1xt[:],
                        w1x[l][:, mc * 128:(mc + 1) * 128]
                        .rearrange("(d p) c -> p d c", p=128))
                    nc.gpsimd.dma_start(
                        w1gt[:],
                        w1g[l][:, mc * 128:(mc + 1) * 128]
                        .rearrange("(d p) c -> p d c", p=128))
                    xh_ps = psg2.tile([128, T_LOC], F32, tag="xh", name="xh")
                    gt_ps = psg2.tile([128, T_LOC], F32, tag="gt", name="gt")
                    for dc in range(8):
                        for (toff, tsz) in TC:
                            nc.tensor.matmul(
                                xh_ps[:, toff:toff + tsz], w1xt[:, dc, :],
                                xn2[dc][:, toff:toff + tsz],
                                start=(dc == 0), stop=(dc == 7))
                            nc.tensor.matmul(
                                gt_ps[:, toff:toff + tsz], w1gt[:, dc, :],
                                xn2[dc][:, toff:toff + tsz],
                                start=(dc == 0), stop=(dc == 7))
                    gel = sqp2.tile([128, T_LOC], F32R, tag="gel", name="gel")
                    nc.scalar.activation(gel[:], gt_ps[:], AF.Gelu)
                    nc.vector.tensor_tensor(
                        out=ffa[mc][:], in0=xh_ps[:], in1=gel[:], op=OP.mult)

                es_ff1.close()
                with tc.tile_pool(name="ps_ff2", bufs=1, space="PSUM") as pst2:
                  for fblk in (range(2) if "ff2" not in skip else []):
                    fps = [pst2.tile([128, T_LOC], F32, tag=f"f2_{i}", name=f"f2_{i}") for i in range(4)]
                    for ic in range(FFC):
                        w2t = ffw.tile([128, DIM], F32R, tag="w2t", name="w2t")
                        nc.gpsimd.dma_start(w2t[:], w2[l, ic * 128:(ic + 1) * 128, :])
                        for i in range(4):
                            fc = fblk * 4 + i
                            for (toff, tsz) in TC:
                                nc.tensor.matmul(
                                    fps[i][:, toff:toff + tsz],
                                    w2t[:, fc * 128:(fc + 1) * 128],
                                    ffa[ic][:, toff:toff + tsz],
                                    start=(ic == 0), stop=(ic == FFC - 1))
                    for i in range(4):
                        fc = fblk * 4 + i
                        nc.vector.tensor_tensor(
                            out=xt[fc][:], in0=xt[fc][:], in1=fps[i][:], op=OP.add)

        with tc.tile_pool(name="fin", bufs=1) as finp, \
             tc.tile_pool(name="sqf", bufs=2) as sqpf:
            xnf = [finp.tile([128, T_LOC], F32R, tag=f"xnf{d}", name=f"xnf{d}") for d in range(8)]
            layer_norm(xt, xnf, gf_row, sqpf)
            for dc in range(8):
                s = sqpf.tile([128, 1], F32, tag="osum", name="osum")
                nc.vector.reduce_sum(s[:], xnf[dc][:], axis=AX.X)
                nc.sync.dma_start(
                    osum[dc * 128:(dc + 1) * 128].rearrange("(p o) -> p o", o=1), s[:])

    _split_multi_waits(nc)
    return nc


def prepare_inputs(inputs, depth=DEPTH):
    import ml_dtypes
    bf16 = ml_dtypes.bfloat16
    f32 = np.float32
    emb = np.asarray(inputs["emb"], f32)
    fus = np.asarray(inputs["fusion_tokens"], f32)
    embt = np.concatenate([emb, fus], axis=0)
    idx_full = np.asarray(inputs["spliced_index"], np.int32)
    dat_full = np.asarray(inputs["spliced_data"], f32)

    wq_full = np.asarray(inputs["Wq"], f32)[:depth]
    wkv = np.asarray(inputs["Wkv"], f32)[:depth]
    wo_full = np.asarray(inputs["Wo"], f32)[:depth]
    w1_full = np.asarray(inputs["Wff1"], f32)[:depth]
    w2_full = np.asarray(inputs["Wff2"], f32)[:depth]
    wk_full = wkv[:, :, :DIM]
    wv_raw = wkv[:, :, DIM:]
    wv_full = np.zeros((depth, DIM, VCOLS), f32)
    wvh = wv_raw.reshape(depth, DIM, NH, DH)
    wv_full.reshape(depth, DIM, NH, DH + 1)[:, :, :, :DH] = wvh
    w1xp = np.zeros((depth, DIM, FF_PAD), f32)
    w1gp = np.zeros((depth, DIM, FF_PAD), f32)
    w1xp[:, :, :FF] = w1_full[:, :, :FF]
    w1gp[:, :, :FF] = w1_full[:, :, FF:]
    w2p = np.zeros((depth, FF_PAD, DIM), f32)
    w2p[:, :FF, :] = w2_full

    g1 = np.asarray(inputs["ln1_g"], f32)[:depth].reshape(depth, 1, DIM)
    g2 = np.asarray(inputs["ln2_g"], f32)[:depth].reshape(depth, 1, DIM)
    gfv = np.asarray(inputs["norm_g"], f32).reshape(1, DIM)

    consts = np.zeros((128, 128), f32)
    consts[:, 0:64] = 1.0
    consts[:, 64] = 1.0 / DIM
    consts[:, 65] = 1e-5
    eye = np.eye(128, dtype=f32)

    shared = dict(consts=consts, eyein=eye,
                  wq=wq_full.astype(bf16), wk=wk_full.astype(bf16),
                  wv=wv_full.astype(bf16), wo=wo_full.astype(bf16),
                  w1x=w1xp.astype(bf16), w1g=w1gp.astype(bf16),
                  w2=w2p.astype(bf16), g1in=g1, g2in=g2, gfin=gfv)

    in_maps = []
    for c in range(8):
        b, r = c // 2, c % 2
        if r == 0:
            idx = idx_full[b, 0:T_LOC].astype(np.int64)
            w = dat_full[b, 0:T_LOC]
        else:
            spl = idx_full[b, T_LOC:N_SPL].astype(np.int64)
            idx = np.concatenate([spl, np.arange(VOCAB, VOCAB + NF, dtype=np.int64)])
            w = np.concatenate([dat_full[b, T_LOC:N_SPL], np.ones(NF, f32)])
        fmask = np.full((128, 1), float(r), f32)
        in_maps.append(dict(shared, tokr=embt[idx].astype(f32),
                            tokw=w.reshape(T_LOC, 1).astype(f32), fmaskin=fmask))
    return in_maps


def epilogue(osums, inputs):
    f64 = np.float64
    pWkv = np.asarray(inputs["pWkv"], f64)
    pWo = np.asarray(inputs["pWo"], f64)
    ret = np.asarray(inputs["return_tokens"], f64)
    lsc = float(np.asarray(inputs["logit_scale_c"]))
    lsf = float(np.asarray(inputs["logit_scale_f"]))

    meantok = np.stack([
        (osums[2 * b].astype(f64) + osums[2 * b + 1].astype(f64)) / T
        for b in range(B)
    ])
    mv = meantok @ pWkv[:, NH * DH:]
    pooled_pre = mv @ pWo
    spliced = pooled_pre + ret[0]
    fusion = pooled_pre + ret[1]

    def closs(a, bv, ls):
        an = a / np.linalg.norm(a, axis=-1, keepdims=True)
        bn = bv / np.linalg.norm(bv, axis=-1, keepdims=True)
        lg = np.exp(ls) * (an @ bn.T)

        def nll(m):
            mx = m.max(-1, keepdims=True)
            lse = mx + np.log(np.exp(m - mx).sum(-1, keepdims=True))
            return -np.diag(m - lse).mean()

        return (nll(lg) + nll(lg.T)) * 0.5

    loss = closs(spliced, spliced, lsc) + closs(spliced, fusion, lsf)
    return np.float32(loss)


def _get_program():
    if "nc" not in _CACHE:
        _CACHE["nc"] = build_program(DEPTH)
    return _CACHE["nc"]


def kernel(**inputs):
    from concourse.bass_utils import run_bass_kernel_spmd

    nc = _get_program()
    in_maps = prepare_inputs(inputs, DEPTH)
    res = run_bass_kernel_spmd(nc, in_maps, list(range(8)))
    osums = [res.results[c]["osum"] for c in range(8)]
    return epilogue(osums, inputs)

